# revision 3
# baseline (speedup 1.0000x reference)
"""Trainium2 Bass kernel for nn_BoxSampler (balanced positive/negative sampler).

Data-parallel over the batch: 8 rows per NeuronCore across 8 cores.

Key insight: the sampler selects, per row, the num_pos positives and
(512-num_pos) negatives with the LARGEST positions under a fixed random
permutation (jax.random key(42), input-independent). Only the permutation
tail can ever be selected, so the device processes the last 24576 permuted
positions per row (7-sigma margin for the spec'd input distributions,
validated host-side with an exact-equality fallback guard).

Device (per core, 8 rows batched across the 128 partitions):
  - DVE prefix scan over i-ordered (permuted-order) tail masks -> running
    candidate counts per partition
  - PE matmuls with constant block-triangular / block-ones matrices ->
    cross-partition offsets, per-row totals, then exact balanced-sampler
    thresholds T_pos / T_neg via fused compare+accumulate counting
  - j-ordered (ascending original index) selection: per-class fused
    compare against the per-row threshold, OR-combined into a 0/1 plane
Host: positions of set bits (order-preserving => already sorted ascending,
matching the reference's stable top_k) decode through the constant
permutation table to original indices. Bit-exact vs the jax reference.
"""
import sys
sys.path.insert(0, '/opt/trn_rl_repo')
import numpy as np
from contextlib import ExitStack

from concourse import bass, tile, bacc
import concourse.mybir as mybir
from concourse.bass_utils import run_bass_kernel_spmd

N = 131072
B = 64
NS = 512
MAXPOS = 128
TAIL = 24576
NEGTAIL = 2048
NCORES = 8
RPC = 8
FD = TAIL // 16      # 1536
NFD = NEGTAIL // 16  # 128
NHALVES = 2
SELN_ON_GPSIMD = False

AL = mybir.AluOpType
DT = mybir.dt

_CONST = None


def get_constants():
    global _CONST
    if _CONST is not None:
        return _CONST
    import jax
    cpu = jax.local_devices(backend='cpu')[0]
    with jax.default_device(cpu):
        keys = jax.random.split(jax.random.key(42), B)
        # must match the reference's vmapped trace (differs from per-row calls)
        perms = np.asarray(jax.vmap(lambda k: jax.random.permutation(k, N))(keys))
    tail_origidx = np.ascontiguousarray(perms[:, N - TAIL:])   # [B, TAIL]
    order = np.argsort(tail_origidx, axis=1).astype(np.int32)  # k -> i
    jval = np.take_along_axis(tail_origidx, order, axis=1)     # ascending js
    g = np.arange(128) // 16
    grp = (g[:, None] == g[None, :]).astype(np.float32)
    tri = grp * (np.arange(128)[:, None] < np.arange(128)[None, :]).astype(np.float32)
    cb = np.zeros((128, 2), np.float32)
    cb[:, 0] = 1.0
    cb[:, 1] = float(TAIL - NEGTAIL + 1)
    _CONST = dict(perms=perms, tail_origidx=tail_origidx, order=order, jval=jval,
                  grp=grp, tri=tri, cb=cb)
    return _CONST


def _emit_body(nc, wpool, ppool, dram, const_tiles):
    pmi_d, nmi_d, ppri_d, npri_d, sel_d = dram
    tri, grp, cb = const_tiles

    # input-dependent loads; i-order (phase 1) first, j-order on other queues
    pmi = wpool.tile([128, FD], DT.uint8, tag="pmi")
    nc.sync.dma_start(out=pmi[:], in_=pmi_d[:])
    nmi = wpool.tile([128, NFD], DT.uint8, tag="nmi")
    nc.sync.dma_start(out=nmi[:], in_=nmi_d[:])
    ppri = wpool.tile([128, FD], DT.uint16, tag="ppri")
    nc.scalar.dma_start(out=ppri[:], in_=ppri_d[:])
    npri = wpool.tile([128, FD], DT.uint16, tag="npri")
    nc.gpsimd.dma_start(out=npri[:], in_=npri_d[:])

    # phase 1: within-partition running candidate counts (i-order)
    cp = wpool.tile([128, FD], DT.float16, tag="cp")
    nc.vector.tensor_tensor_scan(cp[:], pmi[:], pmi[:], 0.0, AL.add, AL.bypass)
    cn = wpool.tile([128, NFD], DT.float16, tag="cn")
    nc.vector.tensor_tensor_scan(cn[:], nmi[:], nmi[:], 0.0, AL.add, AL.bypass)

    lasts = wpool.tile([128, 2], DT.float32, tag="lasts")
    nc.vector.tensor_scalar(lasts[:, 0:1], cp[:, FD - 1:FD], 0.0, None, AL.add)
    nc.vector.tensor_scalar(lasts[:, 1:2], cn[:, NFD - 1:NFD], 0.0, None, AL.add)

    # cross-partition (within each row's 16 partitions) offsets and totals
    offs = ppool.tile([128, 2], DT.float32, tag="offs")
    nc.tensor.matmul(offs[:], tri[:], lasts[:], start=True, stop=True)
    tots = ppool.tile([128, 2], DT.float32, tag="tots")
    nc.tensor.matmul(tots[:], grp[:], lasts[:], start=True, stop=True)

    npos = wpool.tile([128, 1], DT.float32, tag="npos")
    nc.vector.tensor_scalar(npos[:], tots[:, 0:1], float(MAXPOS), None, AL.min)
    thp = wpool.tile([128, 1], DT.float32, tag="thp")
    nc.vector.tensor_scalar(thp[:], tots[:, 0:1], npos[:], offs[:, 0:1],
                            AL.subtract, AL.subtract)
    kneg = wpool.tile([128, 1], DT.float32, tag="kneg")
    nc.vector.tensor_scalar(kneg[:], npos[:], -1.0, float(NS), AL.mult, AL.add)
    thn = wpool.tile([128, 1], DT.float32, tag="thn")
    nc.vector.tensor_scalar(thn[:], tots[:, 1:2], kneg[:], offs[:, 1:2],
                            AL.subtract, AL.subtract)

    # threshold positions = #(running count <= to-skip), summed per row group
    cnts = wpool.tile([128, 2], DT.float32, tag="cnts")
    cmpP = wpool.tile([128, FD], DT.uint8, tag="cmpP")
    nc.vector.tensor_scalar(cmpP[:], cp[:], thp[:], None, AL.is_le,
                            AL.add, accum_out=cnts[:, 0:1])
    cmpN = wpool.tile([128, NFD], DT.uint8, tag="cmpN")
    nc.vector.tensor_scalar(cmpN[:], cn[:], thn[:], None, AL.is_le,
                            AL.add, accum_out=cnts[:, 1:2])

    tposg = ppool.tile([128, 2], DT.float32, tag="tposg")
    nc.tensor.matmul(tposg[:], grp[:], cnts[:], start=True, stop=True)
    tpn1 = wpool.tile([128, 2], DT.float32, tag="tpn1")
    nc.vector.tensor_tensor(tpn1[:], tposg[:], cb[:], AL.add)

    # phase 2: selection in ascending-original-index order -> 0/1 plane
    selP = wpool.tile([128, FD], DT.uint8, tag="selP")
    selN = wpool.tile([128, FD], DT.uint8, tag="selN")
    sel = wpool.tile([128, FD], DT.uint8, tag="sel")
    H = FD // NHALVES
    for h in range(NHALVES):
        sl = slice(h * H, (h + 1) * H)
        nc.vector.tensor_scalar(selP[:, sl], ppri[:, sl], tpn1[:, 0:1], None, AL.is_ge)
        eng = nc.gpsimd if SELN_ON_GPSIMD else nc.vector
        eng.tensor_scalar(selN[:, sl], npri[:, sl], tpn1[:, 1:2], None, AL.is_ge)
        nc.vector.tensor_tensor(sel[:, sl], selP[:, sl], selN[:, sl], AL.logical_or)
        nc.scalar.dma_start(out=sel_d[:, sl], in_=sel[:, sl])


def _build(loop_iters):
    nc = bacc.Bacc("TRN2", target_bir_lowering=False, debug=False, num_devices=NCORES)

    pmi_d = nc.dram_tensor("pmi", [128, FD], DT.uint8, kind="ExternalInput").ap()
    nmi_d = nc.dram_tensor("nmi", [128, NFD], DT.uint8, kind="ExternalInput").ap()
    ppri_d = nc.dram_tensor("ppri", [128, FD], DT.uint16, kind="ExternalInput").ap()
    npri_d = nc.dram_tensor("npri", [128, FD], DT.uint16, kind="ExternalInput").ap()
    tri_d = nc.dram_tensor("tri", [128, 128], DT.float32, kind="ExternalInput").ap()
    grp_d = nc.dram_tensor("grp", [128, 128], DT.float32, kind="ExternalInput").ap()
    cb_d = nc.dram_tensor("cbias", [128, 2], DT.float32, kind="ExternalInput").ap()
    sel_d = nc.dram_tensor("sel", [128, FD], DT.uint8, kind="ExternalOutput").ap()

    with tile.TileContext(nc) as tc:
        with ExitStack() as ctx:
            nb = 2 if loop_iters > 1 else 1
            cpool = ctx.enter_context(tc.tile_pool(name="consts", bufs=1))
            wpool = ctx.enter_context(tc.tile_pool(name="work", bufs=nb))
            ppool = ctx.enter_context(tc.tile_pool(name="psum", bufs=nb, space="PSUM"))

            tri = cpool.tile([128, 128], DT.float32, tag="tri")
            nc.sync.dma_start(out=tri[:], in_=tri_d[:])
            grp = cpool.tile([128, 128], DT.float32, tag="grp")
            nc.sync.dma_start(out=grp[:], in_=grp_d[:])
            cb = cpool.tile([128, 2], DT.float32, tag="cb")
            nc.sync.dma_start(out=cb[:], in_=cb_d[:])

            dram = (pmi_d, nmi_d, ppri_d, npri_d, sel_d)
            for _ in range(loop_iters):
                _emit_body(nc, wpool, ppool, dram, (tri, grp, cb))
    nc.compile()
    return nc


_NC = {}


def build_nc():
    if 1 not in _NC:
        _NC[1] = _build(1)
    return _NC[1]


def build_nc_looped(k):
    if k not in _NC:
        _NC[k] = _build(k)
    return _NC[k]


def host_prepare(positive_matches, negative_matches, ignored_matches):
    C = get_constants()
    posm = positive_matches & ~ignored_matches
    negm = negative_matches & ~positive_matches & ~ignored_matches

    ridx = np.arange(B)[:, None]
    pm_i = posm[ridx, C['tail_origidx']]
    nm_i = negm[ridx, C['tail_origidx']]
    pm_j = posm[ridx, C['jval']]
    nm_j = negm[ridx, C['jval']]

    tpos1 = C['order'] + 1
    ppri = np.where(pm_j, tpos1, 0).astype(np.uint16)
    npri = np.where(nm_j, tpos1, 0).astype(np.uint16)
    ppri_w = np.ascontiguousarray(ppri.reshape(B, FD, 16).transpose(0, 2, 1))
    npri_w = np.ascontiguousarray(npri.reshape(B, FD, 16).transpose(0, 2, 1))

    pmi_l = pm_i.astype(np.uint8).reshape(B, 16, FD)
    nmi_l = nm_i[:, TAIL - NEGTAIL:].astype(np.uint8).reshape(B, 16, NFD)

    in_maps = []
    for c in range(NCORES):
        sl = slice(c * RPC, (c + 1) * RPC)
        in_maps.append({
            "pmi": pmi_l[sl].reshape(128, FD),
            "nmi": nmi_l[sl].reshape(128, NFD),
            "ppri": ppri_w[sl].reshape(128, FD),
            "npri": npri_w[sl].reshape(128, FD),
            "tri": C['tri'],
            "grp": C['grp'],
            "cbias": C['cb'],
        })

    # exactness guards (never trip for the spec'd input distributions)
    npos_tail = pm_i.sum(axis=1)
    npos_tot = posm.sum(axis=1)
    nneg_win = nm_i[:, TAIL - NEGTAIL:].sum(axis=1)
    num_pos = np.minimum(MAXPOS, npos_tot)
    kneg = NS - num_pos
    cand_tot = npos_tot + negm.sum(axis=1)
    safe = (np.all((npos_tail >= MAXPOS) | (npos_tail == npos_tot))
            and np.all(nneg_win >= kneg)
            and np.all(cand_tot >= NS))
    return in_maps, bool(safe)


def unpack_outputs(results):
    C = get_constants()
    out = np.empty((B, NS), np.int32)
    for c in range(NCORES):
        sel = results[c]["sel"]
        selr = sel.reshape(RPC, 16, FD).transpose(0, 2, 1).reshape(RPC, TAIL)
        for r in range(RPC):
            R = c * RPC + r
            ks = np.flatnonzero(selr[r])
            if ks.size != NS:
                return None
            out[R] = C['jval'][R, ks]
    return out


def numpy_reference(positive_matches, negative_matches, ignored_matches):
    """Exact numpy port of the jax reference (fallback for degenerate inputs)."""
    C = get_constants()
    perms = C['perms']
    pos = positive_matches
    cand = (pos | negative_matches) & ~ignored_matches
    out = np.empty((B, NS), np.int32)
    for r in range(B):
        ind = cand[r]
        num_true = ind.sum()
        add_false = np.cumsum(~ind) <= (NS - num_true)
        ind = ind | add_false
        perm = perms[r]
        ind_p = ind[perm]
        lab_p = pos[r][perm]
        idx = np.where(ind_p, np.arange(1, N + 1, dtype=np.int64), 0)
        signed = np.where(lab_p, 1, -1)
        s_idx = idx * signed
        sorted_desc = np.flip(np.sort(s_idx))
        num_pos_avail = int((sorted_desc > 0).sum())
        num_pos = min(MAXPOS, num_pos_avail)
        rk = np.arange(NS)
        gather_idx = np.where(rk < num_pos, rk, N - NS + rk)
        sampled = sorted_desc[gather_idx]
        sampled = np.abs(sampled) - 1
        sampled = np.where(sampled >= 0, sampled, 0)
        ind2 = np.zeros(N, np.int32)
        ind2[sampled] = 1
        o = np.zeros(N, np.int32)
        o[perm] = ind2
        order = np.lexsort((np.arange(N), -o))
        out[r] = order[:NS]
    return out


def kernel(positive_matches, negative_matches, ignored_matches):
    pos = np.asarray(positive_matches)
    neg = np.asarray(negative_matches)
    ign = np.asarray(ignored_matches)
    in_maps, safe = host_prepare(pos, neg, ign)
    if not safe:
        return numpy_reference(pos, neg, ign)
    nc = build_nc()
    res = run_bass_kernel_spmd(nc, in_maps, list(range(NCORES)))
    out = unpack_outputs(res.results)
    if out is None:
        return numpy_reference(pos, neg, ign)
    return out


# revision 6
# speedup vs baseline: 222.0884x; 222.0884x over previous
"""Trainium2 Bass kernel for nn_BoxSampler (balanced positive/negative sampler).

Data-parallel over the batch: 8 rows per NeuronCore across 8 cores.

Key insight: the sampler selects, per row, the num_pos positives and
(512-num_pos) negatives with the LARGEST positions under a fixed random
permutation (jax.random key(42), input-independent). Only the permutation
tail can ever be selected, so the device processes the last 24576 permuted
positions per row (7-sigma margin for the spec'd input distributions,
validated host-side with an exact-equality fallback guard).

Device (per core, 8 rows batched across the 128 partitions):
  - DVE prefix scan over i-ordered (permuted-order) tail masks -> running
    candidate counts per partition
  - PE matmuls with constant block-triangular / block-ones matrices ->
    cross-partition offsets, per-row totals, then exact balanced-sampler
    thresholds T_pos / T_neg via fused compare+accumulate counting
  - j-ordered (ascending original index) selection: per-class fused
    compare against the per-row threshold -> two 0/1 planes
Host: positions of set bits (order-preserving => already sorted ascending,
matching the reference's stable top_k) decode through the constant
permutation table to original indices. Bit-exact vs the jax reference.
"""
import sys
sys.path.insert(0, '/opt/trn_rl_repo')
import numpy as np
from contextlib import ExitStack

from concourse import bass, tile, bacc
import concourse.mybir as mybir
from concourse.bass_utils import run_bass_kernel_spmd

N = 131072
B = 64
NS = 512
MAXPOS = 128
TAIL = 24576
NEGTAIL = 2048
NCORES = 8
RPC = 8
FD = TAIL // 16      # 1536
NFD = NEGTAIL // 16  # 128
NHALVES = 2
SELN_ON_GPSIMD = False

AL = mybir.AluOpType
DT = mybir.dt

_CONST = None


def get_constants():
    global _CONST
    if _CONST is not None:
        return _CONST
    import jax
    cpu = jax.local_devices(backend='cpu')[0]
    with jax.default_device(cpu):
        keys = jax.random.split(jax.random.key(42), B)
        # must match the reference's vmapped trace (differs from per-row calls)
        perms = np.asarray(jax.vmap(lambda k: jax.random.permutation(k, N))(keys))
    tail_origidx = np.ascontiguousarray(perms[:, N - TAIL:])   # [B, TAIL]
    order = np.argsort(tail_origidx, axis=1).astype(np.int32)  # k -> i
    jval = np.take_along_axis(tail_origidx, order, axis=1)     # ascending js
    g = np.arange(128) // 16
    grp = (g[:, None] == g[None, :]).astype(np.float32)
    tri = grp * (np.arange(128)[:, None] < np.arange(128)[None, :]).astype(np.float32)
    cb = np.zeros((128, 2), np.float32)
    cb[:, 0] = 1.0
    cb[:, 1] = float(TAIL - NEGTAIL + 1)
    _CONST = dict(perms=perms, tail_origidx=tail_origidx, order=order, jval=jval,
                  grp=grp, tri=tri, cb=cb)
    return _CONST


def _emit_body(nc, wpool, ppool, dram, const_tiles):
    pmi_d, nmi_d, ppri_d, npri_d, sel_d = dram
    tri, grp, cb = const_tiles

    # input-dependent loads; i-order (phase 1) first, j-order on other queues
    pmi = wpool.tile([128, FD], DT.uint8, tag="pmi")
    nc.sync.dma_start(out=pmi[:], in_=pmi_d[:])
    nmi = wpool.tile([128, NFD], DT.uint8, tag="nmi")
    nc.sync.dma_start(out=nmi[:], in_=nmi_d[:])
    ppri = wpool.tile([128, FD], DT.uint16, tag="ppri")
    nc.scalar.dma_start(out=ppri[:], in_=ppri_d[:])
    npri = wpool.tile([128, FD], DT.uint16, tag="npri")
    nc.gpsimd.dma_start(out=npri[:], in_=npri_d[:])

    # phase 1: within-partition running candidate counts (i-order)
    cp = wpool.tile([128, FD], DT.float16, tag="cp")
    nc.vector.tensor_tensor_scan(cp[:], pmi[:], pmi[:], 0.0, AL.add, AL.bypass)
    cn = wpool.tile([128, NFD], DT.float16, tag="cn")
    nc.vector.tensor_tensor_scan(cn[:], nmi[:], nmi[:], 0.0, AL.add, AL.bypass)

    lasts = wpool.tile([128, 2], DT.float32, tag="lasts")
    nc.vector.tensor_scalar(lasts[:, 0:1], cp[:, FD - 1:FD], 0.0, None, AL.add)
    nc.vector.tensor_scalar(lasts[:, 1:2], cn[:, NFD - 1:NFD], 0.0, None, AL.add)

    # cross-partition (within each row's 16 partitions) offsets and totals
    offs = ppool.tile([128, 2], DT.float32, tag="offs")
    nc.tensor.matmul(offs[:], tri[:], lasts[:], start=True, stop=True)
    tots = ppool.tile([128, 2], DT.float32, tag="tots")
    nc.tensor.matmul(tots[:], grp[:], lasts[:], start=True, stop=True)

    npos = wpool.tile([128, 1], DT.float32, tag="npos")
    nc.vector.tensor_scalar(npos[:], tots[:, 0:1], float(MAXPOS), None, AL.min)
    thp = wpool.tile([128, 1], DT.float32, tag="thp")
    nc.vector.tensor_scalar(thp[:], tots[:, 0:1], npos[:], offs[:, 0:1],
                            AL.subtract, AL.subtract)
    kneg = wpool.tile([128, 1], DT.float32, tag="kneg")
    nc.vector.tensor_scalar(kneg[:], npos[:], -1.0, float(NS), AL.mult, AL.add)
    thn = wpool.tile([128, 1], DT.float32, tag="thn")
    nc.vector.tensor_scalar(thn[:], tots[:, 1:2], kneg[:], offs[:, 1:2],
                            AL.subtract, AL.subtract)

    # threshold positions = #(running count <= to-skip), summed per row group
    cnts = wpool.tile([128, 2], DT.float32, tag="cnts")
    cmpP = wpool.tile([128, FD], DT.uint8, tag="cmpP")
    nc.vector.tensor_scalar(cmpP[:], cp[:], thp[:], None, AL.is_le,
                            AL.add, accum_out=cnts[:, 0:1])
    cmpN = wpool.tile([128, NFD], DT.uint8, tag="cmpN")
    nc.vector.tensor_scalar(cmpN[:], cn[:], thn[:], None, AL.is_le,
                            AL.add, accum_out=cnts[:, 1:2])

    tposg = ppool.tile([128, 2], DT.float32, tag="tposg")
    nc.tensor.matmul(tposg[:], grp[:], cnts[:], start=True, stop=True)
    tpn1 = wpool.tile([128, 2], DT.float32, tag="tpn1")
    nc.vector.tensor_tensor(tpn1[:], tposg[:], cb[:], AL.add)

    # phase 2: selection in ascending-original-index order -> two 0/1 planes
    selP_d, selN_d = sel_d
    selP = wpool.tile([128, FD], DT.uint8, tag="selP")
    selN = wpool.tile([128, FD], DT.uint8, tag="selN")
    H = FD // NHALVES
    for h in range(NHALVES):
        sl = slice(h * H, (h + 1) * H)
        nc.vector.tensor_scalar(selP[:, sl], ppri[:, sl], tpn1[:, 0:1], None, AL.is_ge)
        eng = nc.gpsimd if SELN_ON_GPSIMD else nc.vector
        eng.tensor_scalar(selN[:, sl], npri[:, sl], tpn1[:, 1:2], None, AL.is_ge)
        nc.scalar.dma_start(out=selP_d[:, sl], in_=selP[:, sl])
        nc.scalar.dma_start(out=selN_d[:, sl], in_=selN[:, sl])


def _build(loop_iters):
    nc = bacc.Bacc("TRN2", target_bir_lowering=False, debug=False, num_devices=NCORES)

    pmi_d = nc.dram_tensor("pmi", [128, FD], DT.uint8, kind="ExternalInput").ap()
    nmi_d = nc.dram_tensor("nmi", [128, NFD], DT.uint8, kind="ExternalInput").ap()
    ppri_d = nc.dram_tensor("ppri", [128, FD], DT.uint16, kind="ExternalInput").ap()
    npri_d = nc.dram_tensor("npri", [128, FD], DT.uint16, kind="ExternalInput").ap()
    tri_d = nc.dram_tensor("tri", [128, 128], DT.float32, kind="ExternalInput").ap()
    grp_d = nc.dram_tensor("grp", [128, 128], DT.float32, kind="ExternalInput").ap()
    cb_d = nc.dram_tensor("cbias", [128, 2], DT.float32, kind="ExternalInput").ap()
    selp_d = nc.dram_tensor("selp", [128, FD], DT.uint8, kind="ExternalOutput").ap()
    seln_d = nc.dram_tensor("seln", [128, FD], DT.uint8, kind="ExternalOutput").ap()

    with tile.TileContext(nc) as tc:
        with ExitStack() as ctx:
            nb = 2 if loop_iters > 1 else 1
            cpool = ctx.enter_context(tc.tile_pool(name="consts", bufs=1))
            wpool = ctx.enter_context(tc.tile_pool(name="work", bufs=nb))
            ppool = ctx.enter_context(tc.tile_pool(name="psum", bufs=nb, space="PSUM"))

            tri = cpool.tile([128, 128], DT.float32, tag="tri")
            nc.scalar.dma_start(out=tri[:], in_=tri_d[:])
            grp = cpool.tile([128, 128], DT.float32, tag="grp")
            nc.scalar.dma_start(out=grp[:], in_=grp_d[:])
            cb = cpool.tile([128, 2], DT.float32, tag="cb")
            nc.scalar.dma_start(out=cb[:], in_=cb_d[:])

            dram = (pmi_d, nmi_d, ppri_d, npri_d, (selp_d, seln_d))
            for _ in range(loop_iters):
                _emit_body(nc, wpool, ppool, dram, (tri, grp, cb))
    nc.compile()
    return nc


_NC = {}


def build_nc():
    if 1 not in _NC:
        _NC[1] = _build(1)
    return _NC[1]


def build_nc_looped(k):
    if k not in _NC:
        _NC[k] = _build(k)
    return _NC[k]


def host_prepare(positive_matches, negative_matches, ignored_matches):
    C = get_constants()
    posm = positive_matches & ~ignored_matches
    negm = negative_matches & ~positive_matches & ~ignored_matches

    ridx = np.arange(B)[:, None]
    pm_i = posm[ridx, C['tail_origidx']]
    nm_i = negm[ridx, C['tail_origidx']]
    pm_j = posm[ridx, C['jval']]
    nm_j = negm[ridx, C['jval']]

    tpos1 = C['order'] + 1
    ppri = np.where(pm_j, tpos1, 0).astype(np.uint16)
    npri = np.where(nm_j, tpos1, 0).astype(np.uint16)
    ppri_w = np.ascontiguousarray(ppri.reshape(B, FD, 16).transpose(0, 2, 1))
    npri_w = np.ascontiguousarray(npri.reshape(B, FD, 16).transpose(0, 2, 1))

    pmi_l = pm_i.astype(np.uint8).reshape(B, 16, FD)
    nmi_l = nm_i[:, TAIL - NEGTAIL:].astype(np.uint8).reshape(B, 16, NFD)

    in_maps = []
    for c in range(NCORES):
        sl = slice(c * RPC, (c + 1) * RPC)
        in_maps.append({
            "pmi": pmi_l[sl].reshape(128, FD),
            "nmi": nmi_l[sl].reshape(128, NFD),
            "ppri": ppri_w[sl].reshape(128, FD),
            "npri": npri_w[sl].reshape(128, FD),
            "tri": C['tri'],
            "grp": C['grp'],
            "cbias": C['cb'],
        })

    # exactness guards (never trip for the spec'd input distributions)
    npos_tail = pm_i.sum(axis=1)
    npos_tot = posm.sum(axis=1)
    nneg_win = nm_i[:, TAIL - NEGTAIL:].sum(axis=1)
    num_pos = np.minimum(MAXPOS, npos_tot)
    kneg = NS - num_pos
    cand_tot = npos_tot + negm.sum(axis=1)
    safe = (np.all((npos_tail >= MAXPOS) | (npos_tail == npos_tot))
            and np.all(nneg_win >= kneg)
            and np.all(cand_tot >= NS))
    return in_maps, bool(safe)


def unpack_outputs(results):
    C = get_constants()
    out = np.empty((B, NS), np.int32)
    for c in range(NCORES):
        sel = results[c]["selp"] | results[c]["seln"]
        selr = sel.reshape(RPC, 16, FD).transpose(0, 2, 1).reshape(RPC, TAIL)
        for r in range(RPC):
            R = c * RPC + r
            ks = np.flatnonzero(selr[r])
            if ks.size != NS:
                return None
            out[R] = C['jval'][R, ks]
    return out


def numpy_reference(positive_matches, negative_matches, ignored_matches):
    """Exact numpy port of the jax reference (fallback for degenerate inputs)."""
    C = get_constants()
    perms = C['perms']
    pos = positive_matches
    cand = (pos | negative_matches) & ~ignored_matches
    out = np.empty((B, NS), np.int32)
    for r in range(B):
        ind = cand[r]
        num_true = ind.sum()
        add_false = np.cumsum(~ind) <= (NS - num_true)
        ind = ind | add_false
        perm = perms[r]
        ind_p = ind[perm]
        lab_p = pos[r][perm]
        idx = np.where(ind_p, np.arange(1, N + 1, dtype=np.int64), 0)
        signed = np.where(lab_p, 1, -1)
        s_idx = idx * signed
        sorted_desc = np.flip(np.sort(s_idx))
        num_pos_avail = int((sorted_desc > 0).sum())
        num_pos = min(MAXPOS, num_pos_avail)
        rk = np.arange(NS)
        gather_idx = np.where(rk < num_pos, rk, N - NS + rk)
        sampled = sorted_desc[gather_idx]
        sampled = np.abs(sampled) - 1
        sampled = np.where(sampled >= 0, sampled, 0)
        ind2 = np.zeros(N, np.int32)
        ind2[sampled] = 1
        o = np.zeros(N, np.int32)
        o[perm] = ind2
        order = np.lexsort((np.arange(N), -o))
        out[r] = order[:NS]
    return out


def kernel(positive_matches, negative_matches, ignored_matches):
    pos = np.asarray(positive_matches)
    neg = np.asarray(negative_matches)
    ign = np.asarray(ignored_matches)
    in_maps, safe = host_prepare(pos, neg, ign)
    if not safe:
        return numpy_reference(pos, neg, ign)
    nc = build_nc()
    res = run_bass_kernel_spmd(nc, in_maps, list(range(NCORES)))
    out = unpack_outputs(res.results)
    if out is None:
        return numpy_reference(pos, neg, ign)
    return out


# revision 8
# speedup vs baseline: 224.8813x; 1.0126x over previous
"""Trainium2 Bass kernel for nn_BoxSampler (balanced positive/negative sampler).

Data-parallel over the batch: 8 rows per NeuronCore across 8 cores.

Key insight: the sampler selects, per row, the num_pos positives and
(512-num_pos) negatives with the LARGEST positions under a fixed random
permutation (jax.random key(42), input-independent). Only the permutation
tail can ever be selected, so the device processes the last 24576 permuted
positions per row (7-sigma margin for the spec'd input distributions,
validated host-side with an exact-equality fallback guard).

Device (per core, 8 rows batched across the 128 partitions):
  - DVE prefix scan over i-ordered (permuted-order) tail masks -> running
    candidate counts per partition
  - PE matmuls with constant block-triangular / block-ones matrices ->
    cross-partition offsets, per-row totals, then exact balanced-sampler
    thresholds T_pos / T_neg via fused compare+accumulate counting
  - j-ordered (ascending original index) selection: per-class fused
    compare against the per-row threshold -> two 0/1 planes
Host: positions of set bits (order-preserving => already sorted ascending,
matching the reference's stable top_k) decode through the constant
permutation table to original indices. Bit-exact vs the jax reference.
"""
import sys
sys.path.insert(0, '/opt/trn_rl_repo')
import numpy as np
from contextlib import ExitStack

from concourse import bass, tile, bacc
import concourse.mybir as mybir
from concourse.bass_utils import run_bass_kernel_spmd

N = 131072
B = 64
NS = 512
MAXPOS = 128
TAIL = 24576
NEGTAIL = 2048
NCORES = 8
RPC = 8
FD = TAIL // 16      # 1536
NFD = NEGTAIL // 16  # 128
NHALVES = 2
SELN_ON_GPSIMD = False

AL = mybir.AluOpType
DT = mybir.dt

_CONST = None


def get_constants():
    global _CONST
    if _CONST is not None:
        return _CONST
    import jax
    cpu = jax.local_devices(backend='cpu')[0]
    with jax.default_device(cpu):
        keys = jax.random.split(jax.random.key(42), B)
        # must match the reference's vmapped trace (differs from per-row calls)
        perms = np.asarray(jax.vmap(lambda k: jax.random.permutation(k, N))(keys))
    tail_origidx = np.ascontiguousarray(perms[:, N - TAIL:])   # [B, TAIL]
    order = np.argsort(tail_origidx, axis=1).astype(np.int32)  # k -> i
    jval = np.take_along_axis(tail_origidx, order, axis=1)     # ascending js
    g = np.arange(128) // 16
    grp = (g[:, None] == g[None, :]).astype(np.float32)
    tri = grp * (np.arange(128)[:, None] < np.arange(128)[None, :]).astype(np.float32)
    onesrow = np.zeros((128, 128), np.float32)
    onesrow[0, :] = 1.0
    cb2 = np.zeros((128, 2), np.float32)
    cb2[0, 0] = 1.0
    cb2[0, 1] = float(TAIL - NEGTAIL + 1)
    _CONST = dict(perms=perms, tail_origidx=tail_origidx, order=order, jval=jval,
                  grp=grp, tri=tri, grp16=grp.astype(np.float16),
                  tri16=tri.astype(np.float16), onesrow=onesrow, cb2=cb2)
    return _CONST


def _emit_body(nc, wpool, ppool, dram, const_tiles):
    pmi_d, nmi_d, ppri_d, npri_d, sel_d = dram
    tri, grp, grpf, onesrow, cb = const_tiles

    # input-dependent loads; i-order (phase 1) first, j-order on other queues
    pmi = wpool.tile([128, FD], DT.uint8, tag="pmi")
    nc.sync.dma_start(out=pmi[:], in_=pmi_d[:])
    nmi = wpool.tile([128, NFD], DT.uint8, tag="nmi")
    nc.sync.dma_start(out=nmi[:], in_=nmi_d[:])
    ppri = wpool.tile([128, FD], DT.uint16, tag="ppri")
    nc.scalar.dma_start(out=ppri[:], in_=ppri_d[:])
    npri = wpool.tile([128, FD], DT.uint16, tag="npri")
    nc.gpsimd.dma_start(out=npri[:], in_=npri_d[:])

    # phase 1: within-partition running candidate counts (i-order)
    cp = wpool.tile([128, FD], DT.float16, tag="cp")
    nc.vector.tensor_tensor_scan(cp[:], pmi[:], pmi[:], 0.0, AL.add, AL.bypass)
    cn = wpool.tile([128, NFD], DT.float16, tag="cn")
    nc.vector.tensor_tensor_scan(cn[:], nmi[:], nmi[:], 0.0, AL.add, AL.bypass)

    # cross-partition (within each row's 16 partitions) offsets and totals;
    # rhs reads the scan tails directly (fp16 matmuls, exact for counts<=2048)
    offs = ppool.tile([128, 2], DT.float32, tag="offs")
    nc.tensor.matmul(offs[:, 0:1], tri[:], cp[:, FD - 1:FD], start=True, stop=True)
    nc.tensor.matmul(offs[:, 1:2], tri[:], cn[:, NFD - 1:NFD], start=True, stop=True)
    tots = ppool.tile([128, 2], DT.float32, tag="tots")
    nc.tensor.matmul(tots[:, 0:1], grp[:], cp[:, FD - 1:FD], start=True, stop=True)
    nc.tensor.matmul(tots[:, 1:2], grp[:], cn[:, NFD - 1:NFD], start=True, stop=True)

    npos = wpool.tile([128, 1], DT.float32, tag="npos")
    nc.vector.tensor_scalar(npos[:], tots[:, 0:1], float(MAXPOS), None, AL.min)
    thp = wpool.tile([128, 1], DT.float32, tag="thp")
    nc.vector.tensor_scalar(thp[:], tots[:, 0:1], npos[:], offs[:, 0:1],
                            AL.subtract, AL.subtract)
    kneg = wpool.tile([128, 1], DT.float32, tag="kneg")
    nc.vector.tensor_scalar(kneg[:], npos[:], -1.0, float(NS), AL.mult, AL.add)
    thn = wpool.tile([128, 1], DT.float32, tag="thn")
    nc.vector.tensor_scalar(thn[:], tots[:, 1:2], kneg[:], offs[:, 1:2],
                            AL.subtract, AL.subtract)

    # threshold positions = #(running count <= to-skip), summed per row group
    cnts = wpool.tile([128, 2], DT.float32, tag="cnts")
    cmpP = wpool.tile([128, FD], DT.uint8, tag="cmpP")
    nc.vector.tensor_scalar(cmpP[:], cp[:], thp[:], None, AL.is_le,
                            AL.add, accum_out=cnts[:, 0:1])
    cmpN = wpool.tile([128, NFD], DT.uint8, tag="cmpN")
    nc.vector.tensor_scalar(cmpN[:], cn[:], thn[:], None, AL.is_le,
                            AL.add, accum_out=cnts[:, 1:2])

    # threshold positions + (1, TAIL-NEGTAIL+1) bias fused into one PSUM group
    tpn1 = ppool.tile([128, 2], DT.float32, tag="tpn1")
    nc.tensor.matmul(tpn1[:], grpf[:], cnts[:], start=True, stop=False)
    nc.tensor.matmul(tpn1[:], onesrow[:], cb[:], start=False, stop=True)

    # phase 2: selection in ascending-original-index order -> two 0/1 planes
    selP_d, selN_d = sel_d
    selP = wpool.tile([128, FD], DT.uint8, tag="selP")
    selN = wpool.tile([128, FD], DT.uint8, tag="selN")
    H = FD // NHALVES
    for h in range(NHALVES):
        sl = slice(h * H, (h + 1) * H)
        nc.vector.tensor_scalar(selP[:, sl], ppri[:, sl], tpn1[:, 0:1], None, AL.is_ge)
        eng = nc.gpsimd if SELN_ON_GPSIMD else nc.vector
        eng.tensor_scalar(selN[:, sl], npri[:, sl], tpn1[:, 1:2], None, AL.is_ge)
        nc.scalar.dma_start(out=selP_d[:, sl], in_=selP[:, sl])
        nc.scalar.dma_start(out=selN_d[:, sl], in_=selN[:, sl])


def _build(loop_iters):
    nc = bacc.Bacc("TRN2", target_bir_lowering=False, debug=False, num_devices=NCORES)

    pmi_d = nc.dram_tensor("pmi", [128, FD], DT.uint8, kind="ExternalInput").ap()
    nmi_d = nc.dram_tensor("nmi", [128, NFD], DT.uint8, kind="ExternalInput").ap()
    ppri_d = nc.dram_tensor("ppri", [128, FD], DT.uint16, kind="ExternalInput").ap()
    npri_d = nc.dram_tensor("npri", [128, FD], DT.uint16, kind="ExternalInput").ap()
    tri_d = nc.dram_tensor("tri16", [128, 128], DT.float16, kind="ExternalInput").ap()
    grp_d = nc.dram_tensor("grp16", [128, 128], DT.float16, kind="ExternalInput").ap()
    grpf_d = nc.dram_tensor("grp", [128, 128], DT.float32, kind="ExternalInput").ap()
    or_d = nc.dram_tensor("onesrow", [128, 128], DT.float32, kind="ExternalInput").ap()
    cb_d = nc.dram_tensor("cb2", [128, 2], DT.float32, kind="ExternalInput").ap()
    selp_d = nc.dram_tensor("selp", [128, FD], DT.uint8, kind="ExternalOutput").ap()
    seln_d = nc.dram_tensor("seln", [128, FD], DT.uint8, kind="ExternalOutput").ap()

    with tile.TileContext(nc) as tc:
        with ExitStack() as ctx:
            nb = 2 if loop_iters > 1 else 1
            cpool = ctx.enter_context(tc.tile_pool(name="consts", bufs=1))
            wpool = ctx.enter_context(tc.tile_pool(name="work", bufs=nb))
            ppool = ctx.enter_context(tc.tile_pool(name="psum", bufs=nb, space="PSUM"))

            tri = cpool.tile([128, 128], DT.float16, tag="tri")
            nc.scalar.dma_start(out=tri[:], in_=tri_d[:])
            grp = cpool.tile([128, 128], DT.float16, tag="grp")
            nc.scalar.dma_start(out=grp[:], in_=grp_d[:])
            grpf = cpool.tile([128, 128], DT.float32, tag="grpf")
            nc.scalar.dma_start(out=grpf[:], in_=grpf_d[:])
            onesrow = cpool.tile([128, 128], DT.float32, tag="onesrow")
            nc.scalar.dma_start(out=onesrow[:], in_=or_d[:])
            cb = cpool.tile([128, 2], DT.float32, tag="cb")
            nc.scalar.dma_start(out=cb[:], in_=cb_d[:])

            dram = (pmi_d, nmi_d, ppri_d, npri_d, (selp_d, seln_d))
            for _ in range(loop_iters):
                _emit_body(nc, wpool, ppool, dram, (tri, grp, grpf, onesrow, cb))
    nc.compile()
    return nc


_NC = {}


def build_nc():
    if 1 not in _NC:
        _NC[1] = _build(1)
    return _NC[1]


def build_nc_looped(k):
    if k not in _NC:
        _NC[k] = _build(k)
    return _NC[k]


def host_prepare(positive_matches, negative_matches, ignored_matches):
    C = get_constants()
    posm = positive_matches & ~ignored_matches
    negm = negative_matches & ~positive_matches & ~ignored_matches

    ridx = np.arange(B)[:, None]
    pm_i = posm[ridx, C['tail_origidx']]
    nm_i = negm[ridx, C['tail_origidx']]
    pm_j = posm[ridx, C['jval']]
    nm_j = negm[ridx, C['jval']]

    tpos1 = C['order'] + 1
    ppri = np.where(pm_j, tpos1, 0).astype(np.uint16)
    npri = np.where(nm_j, tpos1, 0).astype(np.uint16)
    ppri_w = np.ascontiguousarray(ppri.reshape(B, FD, 16).transpose(0, 2, 1))
    npri_w = np.ascontiguousarray(npri.reshape(B, FD, 16).transpose(0, 2, 1))

    pmi_l = pm_i.astype(np.uint8).reshape(B, 16, FD)
    nmi_l = nm_i[:, TAIL - NEGTAIL:].astype(np.uint8).reshape(B, 16, NFD)

    in_maps = []
    for c in range(NCORES):
        sl = slice(c * RPC, (c + 1) * RPC)
        in_maps.append({
            "pmi": pmi_l[sl].reshape(128, FD),
            "nmi": nmi_l[sl].reshape(128, NFD),
            "ppri": ppri_w[sl].reshape(128, FD),
            "npri": npri_w[sl].reshape(128, FD),
            "tri16": C['tri16'],
            "grp16": C['grp16'],
            "grp": C['grp'],
            "onesrow": C['onesrow'],
            "cb2": C['cb2'],
        })

    # exactness guards (never trip for the spec'd input distributions)
    npos_tail = pm_i.sum(axis=1)
    npos_tot = posm.sum(axis=1)
    nneg_win = nm_i[:, TAIL - NEGTAIL:].sum(axis=1)
    num_pos = np.minimum(MAXPOS, npos_tot)
    kneg = NS - num_pos
    cand_tot = npos_tot + negm.sum(axis=1)
    safe = (np.all((npos_tail >= MAXPOS) | (npos_tail == npos_tot))
            and np.all(nneg_win >= kneg)
            and np.all(cand_tot >= NS))
    return in_maps, bool(safe)


def unpack_outputs(results):
    C = get_constants()
    out = np.empty((B, NS), np.int32)
    for c in range(NCORES):
        sel = results[c]["selp"] | results[c]["seln"]
        selr = sel.reshape(RPC, 16, FD).transpose(0, 2, 1).reshape(RPC, TAIL)
        for r in range(RPC):
            R = c * RPC + r
            ks = np.flatnonzero(selr[r])
            if ks.size != NS:
                return None
            out[R] = C['jval'][R, ks]
    return out


def numpy_reference(positive_matches, negative_matches, ignored_matches):
    """Exact numpy port of the jax reference (fallback for degenerate inputs)."""
    C = get_constants()
    perms = C['perms']
    pos = positive_matches
    cand = (pos | negative_matches) & ~ignored_matches
    out = np.empty((B, NS), np.int32)
    for r in range(B):
        ind = cand[r]
        num_true = ind.sum()
        add_false = np.cumsum(~ind) <= (NS - num_true)
        ind = ind | add_false
        perm = perms[r]
        ind_p = ind[perm]
        lab_p = pos[r][perm]
        idx = np.where(ind_p, np.arange(1, N + 1, dtype=np.int64), 0)
        signed = np.where(lab_p, 1, -1)
        s_idx = idx * signed
        sorted_desc = np.flip(np.sort(s_idx))
        num_pos_avail = int((sorted_desc > 0).sum())
        num_pos = min(MAXPOS, num_pos_avail)
        rk = np.arange(NS)
        gather_idx = np.where(rk < num_pos, rk, N - NS + rk)
        sampled = sorted_desc[gather_idx]
        sampled = np.abs(sampled) - 1
        sampled = np.where(sampled >= 0, sampled, 0)
        ind2 = np.zeros(N, np.int32)
        ind2[sampled] = 1
        o = np.zeros(N, np.int32)
        o[perm] = ind2
        order = np.lexsort((np.arange(N), -o))
        out[r] = order[:NS]
    return out


def kernel(positive_matches, negative_matches, ignored_matches):
    pos = np.asarray(positive_matches)
    neg = np.asarray(negative_matches)
    ign = np.asarray(ignored_matches)
    in_maps, safe = host_prepare(pos, neg, ign)
    if not safe:
        return numpy_reference(pos, neg, ign)
    nc = build_nc()
    res = run_bass_kernel_spmd(nc, in_maps, list(range(NCORES)))
    out = unpack_outputs(res.results)
    if out is None:
        return numpy_reference(pos, neg, ign)
    return out
